# revision 2
# baseline (speedup 1.0000x reference)
"""LoRA layer kernel for Trainium2 (Bass/Tile), data-parallel over 8 NeuronCores.

Math:  out = (x @ B) @ A * (32/16)   with x [4,2048,4096], B [4096,16], A [16,4096].

Strategy:
  - Flatten tokens (4*2048=8192), shard 1024 tokens per core (data parallel).
  - Host-side layout prep per shard: feed the device x TRANSPOSED
    (xT [4096, 1024], contiguous) so the contraction dim lands on SBUF
    partitions with perfectly contiguous DMA and no on-chip transpose.
  - B is fed as [128, 32, 16] (i-major chunks on partitions) so each
    contraction chunk is a ready-made lhsT tile. A is pre-scaled by 2.0.
  - mm1: xbT[16, t] = sum_c B_c[128,16].T @ xT_c[128,t]  (PSUM accumulate)
  - mm2: out[t, o] = xbT[:, t-tile].T(lhsT) @ A[16, o-tile]  -> natural
    output layout, contiguous stores.
"""

import os
import numpy as np

IN = 4096
OUT = 4096
R = 16
N_CORES = 8
SCALE = 32.0 / 16.0
P = 128
NB = IN // P  # 32 contraction chunks


def _install_profile_hook():
    """Best-effort: register the axon NTFF profiling hook that this image's
    `antenv` package is missing, so run_bass_kernel_spmd(trace=True) can
    return exec_time_ns. Harmless no-op when anything is unavailable."""
    try:
        import sys
        import types

        if "antenv.axon_hooks" in sys.modules:
            return
        try:
            import antenv  # noqa: F401
        except ImportError:
            return
        mod = types.ModuleType("antenv.axon_hooks")
        mod._hook = None

        def set_axon_ntff_profile_hook(h):
            mod._hook = h

        def get_axon_ntff_profile_hook():
            return mod._hook

        mod.set_axon_ntff_profile_hook = set_axon_ntff_profile_hook
        mod.get_axon_ntff_profile_hook = get_axon_ntff_profile_hook
        sys.modules["antenv.axon_hooks"] = mod
        import antenv as _antenv

        _antenv.axon_hooks = mod

        so_path = "/opt/axon/libaxon_pjrt.so"
        if os.path.exists(so_path):
            try:
                from trn_agent_boot.trn_boot import _ntff_profile_via_ctypes

                hook = _ntff_profile_via_ctypes(so_path)
                if hook is not None:
                    mod._hook = hook
            except Exception:
                pass
    except Exception:
        pass


_install_profile_hook()

_NC_CACHE = {}


def build_nc(tok, tb=256):
    """Build + compile the per-core Bass program for `tok` tokens/core."""
    key = (tok, tb)
    if key in _NC_CACHE:
        return _NC_CACHE[key]

    import concourse.bacc as bacc
    import concourse.tile as tile
    from concourse import mybir

    f32 = mybir.dt.float32
    tb = min(tb, tok)
    assert tok % tb == 0 and tb % P == 0

    nc = bacc.Bacc("TRN2", target_bir_lowering=False, debug=False)
    xT = nc.dram_tensor("xT", [IN, tok], f32, kind="ExternalInput").ap()
    Bt = nc.dram_tensor("Bt", [P, NB, R], f32, kind="ExternalInput").ap()
    At = nc.dram_tensor("At", [R, OUT], f32, kind="ExternalInput").ap()
    out = nc.dram_tensor("out", [tok, OUT], f32, kind="ExternalOutput").ap()

    with tile.TileContext(nc) as tc:
        with (
            tc.tile_pool(name="const", bufs=1) as const_pool,
            tc.tile_pool(name="xin", bufs=2) as x_pool,
            tc.tile_pool(name="xbt", bufs=2) as xbt_pool,
            tc.tile_pool(name="ps1", bufs=2, space="PSUM") as ps1,
            tc.tile_pool(name="ps2", bufs=4, space="PSUM") as ps2,
            tc.tile_pool(name="osb", bufs=3) as out_pool,
        ):
            B_sb = const_pool.tile([P, NB, R], f32)
            nc.sync.dma_start(out=B_sb[:], in_=Bt[:])
            A_sb = const_pool.tile([R, OUT], f32)
            nc.sync.dma_start(out=A_sb[:], in_=At[:])

            for tbi in range(tok // tb):
                # load xT block: [128 part (i%), NB chunks, tb tokens]
                xT_sb = x_pool.tile([P, NB, tb], f32)
                for c in range(NB):
                    nc.sync.dma_start(
                        out=xT_sb[:, c, :],
                        in_=xT[c * P : (c + 1) * P, tbi * tb : (tbi + 1) * tb],
                    )
                # mm1: accumulate xbT[16, tb] over 32 contraction chunks
                ps_xbt = ps1.tile([R, tb], f32)
                for c in range(NB):
                    nc.tensor.matmul(
                        ps_xbt[:],
                        lhsT=B_sb[:, c, :],
                        rhs=xT_sb[:, c, :],
                        start=(c == 0),
                        stop=(c == NB - 1),
                    )
                xbt_sb = xbt_pool.tile([R, tb], f32)
                nc.any.tensor_copy(xbt_sb[:], ps_xbt[:])

                # mm2: per 128-token subtile, 8 x [128,512] output chunks
                for st in range(tb // P):
                    t0 = tbi * tb + st * P
                    o_sb = out_pool.tile([P, OUT], f32)
                    for o in range(OUT // 512):
                        ps_o = ps2.tile([P, 512], f32)
                        nc.tensor.matmul(
                            ps_o[:],
                            lhsT=xbt_sb[:, st * P : (st + 1) * P],
                            rhs=A_sb[:, o * 512 : (o + 1) * 512],
                            start=True,
                            stop=True,
                        )
                        nc.any.tensor_copy(o_sb[:, o * 512 : (o + 1) * 512], ps_o[:])
                    nc.sync.dma_start(out=out[t0 : t0 + P, :], in_=o_sb[:])

    nc.compile()
    _NC_CACHE[key] = nc
    return nc


def make_in_maps(x, lora_A, lora_B, n_cores=N_CORES):
    x = np.asarray(x, dtype=np.float32)
    A = np.asarray(lora_A, dtype=np.float32)
    B = np.asarray(lora_B, dtype=np.float32)
    xf = x.reshape(-1, IN)
    ntok = xf.shape[0] // n_cores
    A_scaled = np.ascontiguousarray(A * np.float32(SCALE))
    B_resh = np.ascontiguousarray(B.reshape(NB, P, R).transpose(1, 0, 2))
    in_maps = []
    for c in range(n_cores):
        shard = xf[c * ntok : (c + 1) * ntok]
        in_maps.append(
            {
                "xT": np.ascontiguousarray(shard.T),
                "Bt": B_resh,
                "At": A_scaled,
            }
        )
    return in_maps, ntok


def kernel_with_results(x, lora_A, lora_B, trace=False, **kwargs):
    from concourse.bass_utils import run_bass_kernel_spmd

    in_maps, ntok = make_in_maps(x, lora_A, lora_B)
    nc = build_nc(ntok)
    res = run_bass_kernel_spmd(nc, in_maps, list(range(N_CORES)), trace=trace, **kwargs)
    out = np.concatenate([r["out"] for r in res.results], axis=0)
    return out.reshape(np.asarray(x).shape[:-1] + (OUT,)), res


def kernel(x, lora_A, lora_B):
    out, _ = kernel_with_results(x, lora_A, lora_B)
    return out


# revision 8
# speedup vs baseline: 1.2847x; 1.2847x over previous
"""LoRA layer kernel for Trainium2 (Bass/Tile), data-parallel over 8 NeuronCores.

Math:  out = (x @ B) @ A * (32/16)   with x [4,2048,4096], B [4096,16], A [16,4096].

Strategy:
  - Flatten tokens (4*2048=8192), shard 1024 tokens per core (data parallel).
  - Host-side layout prep per shard: feed the device x TRANSPOSED
    (xT [4096, 1024], contiguous) so the contraction dim lands on SBUF
    partitions with perfectly contiguous DMA and no on-chip transpose.
  - B is fed as [128, 32, 16] (i-major chunks on partitions) so each
    contraction chunk is a ready-made lhsT tile. A is pre-scaled by 2.0.
  - mm1: xbT[16, t] = sum_c B_c[128,16].T @ xT_c[128,t]  (PSUM accumulate)
  - mm2: out[t, o] = xbT[:, t-tile].T(lhsT) @ A[16, o-tile]  -> natural
    output layout, contiguous stores.
"""

import os
import numpy as np

IN = 4096
OUT = 4096
R = 16
N_CORES = 8
SCALE = 32.0 / 16.0
P = 128
NB = IN // P  # 32 contraction chunks


def _install_profile_hook():
    """Best-effort: register the axon NTFF profiling hook that this image's
    `antenv` package is missing, so run_bass_kernel_spmd(trace=True) can
    return exec_time_ns. Harmless no-op when anything is unavailable."""
    try:
        import sys
        import types

        if "antenv.axon_hooks" in sys.modules:
            return
        try:
            import antenv  # noqa: F401
        except ImportError:
            return
        mod = types.ModuleType("antenv.axon_hooks")
        mod._hook = None

        def set_axon_ntff_profile_hook(h):
            mod._hook = h

        def get_axon_ntff_profile_hook():
            return mod._hook

        mod.set_axon_ntff_profile_hook = set_axon_ntff_profile_hook
        mod.get_axon_ntff_profile_hook = get_axon_ntff_profile_hook
        sys.modules["antenv.axon_hooks"] = mod
        import antenv as _antenv

        _antenv.axon_hooks = mod

        so_path = "/opt/axon/libaxon_pjrt.so"
        if os.path.exists(so_path):
            try:
                from trn_agent_boot.trn_boot import _ntff_profile_via_ctypes

                hook = _ntff_profile_via_ctypes(so_path)
                if hook is not None:
                    mod._hook = hook
            except Exception:
                pass
    except Exception:
        pass


_install_profile_hook()

_NC_CACHE = {}


def build_nc(tok, tb=256):
    """Build + compile the per-core Bass program for `tok` tokens/core."""
    key = (tok, tb)
    if key in _NC_CACHE:
        return _NC_CACHE[key]

    import concourse.bacc as bacc
    import concourse.tile as tile
    from concourse import mybir

    f32 = mybir.dt.float32
    f32r = mybir.dt.float32r  # full-rate PE streaming (1 cyc/row at N>=256)
    tb = min(tb, tok)
    assert tok % tb == 0 and tb % P == 0

    nc = bacc.Bacc("TRN2", target_bir_lowering=False, debug=False)
    xT = nc.dram_tensor("xT", [IN, tok], f32r, kind="ExternalInput").ap()
    Bt = nc.dram_tensor("Bt", [P, NB, R], f32r, kind="ExternalInput").ap()
    At = nc.dram_tensor("At", [R, OUT], f32r, kind="ExternalInput").ap()
    out = nc.dram_tensor("out", [tok, OUT], f32, kind="ExternalOutput").ap()

    with tile.TileContext(nc) as tc:
        with (
            tc.tile_pool(name="const", bufs=1) as const_pool,
            tc.tile_pool(name="xin", bufs=2) as x_pool,
            tc.tile_pool(name="xbt", bufs=2) as xbt_pool,
            tc.tile_pool(name="ps1", bufs=2, space="PSUM") as ps1,
            tc.tile_pool(name="ps2", bufs=4, space="PSUM") as ps2,
            tc.tile_pool(name="osb", bufs=3) as out_pool,
        ):
            B_sb = const_pool.tile([P, NB, R], f32r)
            nc.sync.dma_start(out=B_sb[:], in_=Bt[:])
            A_sb = const_pool.tile([R, OUT], f32r)
            nc.sync.dma_start(out=A_sb[:], in_=At[:])

            for tbi in range(tok // tb):
                # load xT block: [128 part (i%), NB chunks, tb tokens]
                xT_sb = x_pool.tile([P, NB, tb], f32r)
                for c in range(NB):
                    nc.sync.dma_start(
                        out=xT_sb[:, c, :],
                        in_=xT[c * P : (c + 1) * P, tbi * tb : (tbi + 1) * tb],
                    )
                # mm1: accumulate xbT[16, tb] over 32 contraction chunks
                ps_xbt = ps1.tile([R, tb], f32)
                for c in range(NB):
                    nc.tensor.matmul(
                        ps_xbt[:],
                        lhsT=B_sb[:, c, :],
                        rhs=xT_sb[:, c, :],
                        start=(c == 0),
                        stop=(c == NB - 1),
                    )
                xbt_sb = xbt_pool.tile([R, tb], f32r)
                nc.any.tensor_copy(xbt_sb[:], ps_xbt[:])

                # mm2: per 128-token subtile, 8 x [128,512] output chunks
                for st in range(tb // P):
                    t0 = tbi * tb + st * P
                    o_sb = out_pool.tile([P, OUT], f32)
                    for o in range(OUT // 512):
                        ps_o = ps2.tile([P, 512], f32)
                        nc.tensor.matmul(
                            ps_o[:],
                            lhsT=xbt_sb[:, st * P : (st + 1) * P],
                            rhs=A_sb[:, o * 512 : (o + 1) * 512],
                            start=True,
                            stop=True,
                        )
                        # split PSUM->SBUF copies across DVE and ACT
                        if o % 2 == 0:
                            nc.vector.tensor_copy(
                                o_sb[:, o * 512 : (o + 1) * 512], ps_o[:]
                            )
                        else:
                            nc.scalar.activation(
                                o_sb[:, o * 512 : (o + 1) * 512],
                                ps_o[:],
                                mybir.ActivationFunctionType.Copy,
                            )
                    nc.sync.dma_start(out=out[t0 : t0 + P, :], in_=o_sb[:])

    nc.compile()
    _NC_CACHE[key] = nc
    return nc


def make_in_maps(x, lora_A, lora_B, n_cores=N_CORES):
    x = np.asarray(x, dtype=np.float32)
    A = np.asarray(lora_A, dtype=np.float32)
    B = np.asarray(lora_B, dtype=np.float32)
    xf = x.reshape(-1, IN)
    ntok = xf.shape[0] // n_cores
    A_scaled = np.ascontiguousarray(A * np.float32(SCALE))
    B_resh = np.ascontiguousarray(B.reshape(NB, P, R).transpose(1, 0, 2))
    in_maps = []
    for c in range(n_cores):
        shard = xf[c * ntok : (c + 1) * ntok]
        in_maps.append(
            {
                "xT": np.ascontiguousarray(shard.T),
                "Bt": B_resh,
                "At": A_scaled,
            }
        )
    return in_maps, ntok


def kernel_with_results(x, lora_A, lora_B, trace=False, **kwargs):
    from concourse.bass_utils import run_bass_kernel_spmd

    in_maps, ntok = make_in_maps(x, lora_A, lora_B)
    nc = build_nc(ntok)
    res = run_bass_kernel_spmd(nc, in_maps, list(range(N_CORES)), trace=trace, **kwargs)
    out = np.concatenate([r["out"] for r in res.results], axis=0)
    return out.reshape(np.asarray(x).shape[:-1] + (OUT,)), res


def kernel(x, lora_A, lora_B):
    out, _ = kernel_with_results(x, lora_A, lora_B)
    return out


# revision 12
# speedup vs baseline: 1.9088x; 1.4859x over previous
"""LoRA layer kernel for Trainium2 (Bass/Tile), data-parallel over 8 NeuronCores.

Math:  out = (x @ B) @ A * (32/16)   with x [4,2048,4096], B [4096,16], A [16,4096].

Strategy:
  - Flatten tokens (4*2048=8192), shard 1024 tokens per core (data parallel).
  - Host-side layout prep per shard: feed the device x TRANSPOSED
    (xT [4096, 1024], contiguous) so the contraction dim lands on SBUF
    partitions with perfectly contiguous DMA and no on-chip transpose.
  - B is fed as [128, 32, 16] (i-major chunks on partitions) so each
    contraction chunk is a ready-made lhsT tile. A is pre-scaled by 2.0.
  - mm1: xbT[16, t] = sum_c B_c[128,16].T @ xT_c[128,t]  (PSUM accumulate)
  - mm2: out[t, o] = xbT[:, t-tile].T(lhsT) @ A[16, o-tile]  -> natural
    output layout, contiguous stores.
"""

import os
import numpy as np

IN = 4096
OUT = 4096
R = 16
N_CORES = 8
SCALE = 32.0 / 16.0
P = 128
NB = IN // P  # 32 contraction chunks


def _install_profile_hook():
    """Best-effort: register the axon NTFF profiling hook that this image's
    `antenv` package is missing, so run_bass_kernel_spmd(trace=True) can
    return exec_time_ns. Harmless no-op when anything is unavailable."""
    try:
        import sys
        import types

        if "antenv.axon_hooks" in sys.modules:
            return
        try:
            import antenv  # noqa: F401
        except ImportError:
            return
        mod = types.ModuleType("antenv.axon_hooks")
        mod._hook = None

        def set_axon_ntff_profile_hook(h):
            mod._hook = h

        def get_axon_ntff_profile_hook():
            return mod._hook

        mod.set_axon_ntff_profile_hook = set_axon_ntff_profile_hook
        mod.get_axon_ntff_profile_hook = get_axon_ntff_profile_hook
        sys.modules["antenv.axon_hooks"] = mod
        import antenv as _antenv

        _antenv.axon_hooks = mod

        so_path = "/opt/axon/libaxon_pjrt.so"
        if os.path.exists(so_path):
            try:
                from trn_agent_boot.trn_boot import _ntff_profile_via_ctypes

                hook = _ntff_profile_via_ctypes(so_path)
                if hook is not None:
                    mod._hook = hook
            except Exception:
                pass
    except Exception:
        pass


_install_profile_hook()

_NC_CACHE = {}


def build_nc(tok, tb=512, load_split=16):
    """Build + compile the per-core Bass program for `tok` tokens/core.

    x arrives pre-tiled on the host as [tok//tb, NB, 128, tb] so that every
    load descriptor reads a fully contiguous DRAM range.
    """
    key = (tok, tb)
    if key in _NC_CACHE:
        return _NC_CACHE[key]

    import concourse.bacc as bacc
    import concourse.tile as tile
    from concourse import mybir

    f32 = mybir.dt.float32
    f32r = mybir.dt.float32r  # full-rate PE streaming (1 cyc/row at N>=256)
    f16 = mybir.dt.float16  # halves x DMA bytes; mm1 in fp16 (~3e-4 rel err)
    tb = min(tb, tok)
    assert tok % tb == 0 and tb % P == 0
    ntb = tok // tb
    load_split = min(load_split, NB)

    nc = bacc.Bacc("TRN2", target_bir_lowering=False, debug=False)
    xT = nc.dram_tensor("xT", [ntb, NB, P, tb], f16, kind="ExternalInput").ap()
    Bt = nc.dram_tensor("Bt", [P, NB, R], f16, kind="ExternalInput").ap()
    At = nc.dram_tensor("At", [R, OUT], f32r, kind="ExternalInput").ap()
    out = nc.dram_tensor("out", [tok, OUT], f32, kind="ExternalOutput").ap()

    with tile.TileContext(nc) as tc:
        with (
            tc.tile_pool(name="const", bufs=1) as const_pool,
            tc.tile_pool(name="xin", bufs=2) as x_pool,
            tc.tile_pool(name="xbt", bufs=2) as xbt_pool,
            tc.tile_pool(name="ps1", bufs=2, space="PSUM") as ps1,
            tc.tile_pool(name="ps2", bufs=4, space="PSUM") as ps2,
            tc.tile_pool(name="osb", bufs=2) as out_pool,
        ):
            B_sb = const_pool.tile([P, NB, R], f16)
            nc.sync.dma_start(out=B_sb[:], in_=Bt[:])
            A_sb = const_pool.tile([R, OUT], f32r)
            nc.sync.dma_start(out=A_sb[:], in_=At[:])

            cpl = NB // load_split  # chunks per load descriptor
            for tbi in range(ntb):
                # load xT block: [128 part, NB chunks, tb tokens]; each
                # descriptor covers `cpl` chunks = fully contiguous DRAM
                xT_sb = x_pool.tile([P, NB, tb], f16)
                for li in range(load_split):
                    nc.sync.dma_start(
                        out=xT_sb[:, li * cpl : (li + 1) * cpl, :],
                        in_=xT[tbi, li * cpl : (li + 1) * cpl, :, :].rearrange(
                            "c p t -> p c t"
                        ),
                    )
                # mm1: accumulate xbT[16, tb] over 32 contraction chunks
                ps_xbt = ps1.tile([R, tb], f32)
                for c in range(NB):
                    nc.tensor.matmul(
                        ps_xbt[:],
                        lhsT=B_sb[:, c, :],
                        rhs=xT_sb[:, c, :],
                        start=(c == 0),
                        stop=(c == NB - 1),
                    )
                xbt_sb = xbt_pool.tile([R, tb], f32r)
                nc.any.tensor_copy(xbt_sb[:], ps_xbt[:])

                # mm2: per 128-token subtile, 8 x [128,512] output chunks
                for st in range(tb // P):
                    t0 = tbi * tb + st * P
                    o_sb = out_pool.tile([P, OUT], f32)
                    for o in range(OUT // 512):
                        ps_o = ps2.tile([P, 512], f32)
                        nc.tensor.matmul(
                            ps_o[:],
                            lhsT=xbt_sb[:, st * P : (st + 1) * P],
                            rhs=A_sb[:, o * 512 : (o + 1) * 512],
                            start=True,
                            stop=True,
                        )
                        # split PSUM->SBUF copies across DVE and ACT
                        if o % 2 == 0:
                            nc.vector.tensor_copy(
                                o_sb[:, o * 512 : (o + 1) * 512], ps_o[:]
                            )
                        else:
                            nc.scalar.activation(
                                o_sb[:, o * 512 : (o + 1) * 512],
                                ps_o[:],
                                mybir.ActivationFunctionType.Copy,
                            )
                    nc.sync.dma_start(out=out[t0 : t0 + P, :], in_=o_sb[:])

    nc.compile()
    _NC_CACHE[key] = nc
    return nc


TB = 512


def make_in_maps(x, lora_A, lora_B, n_cores=N_CORES):
    x = np.asarray(x, dtype=np.float32)
    A = np.asarray(lora_A, dtype=np.float32)
    B = np.asarray(lora_B, dtype=np.float32)
    xf = x.reshape(-1, IN)
    ntok = xf.shape[0] // n_cores
    tb = min(TB, ntok)
    A_scaled = np.ascontiguousarray(A * np.float32(SCALE))
    B_resh = np.ascontiguousarray(B.reshape(NB, P, R).transpose(1, 0, 2), dtype=np.float16)
    in_maps = []
    for c in range(n_cores):
        shard = xf[c * ntok : (c + 1) * ntok]
        # pre-tile: [ntb, NB, 128, tb]; xT[tbi,c,p,t] = shard[tbi*tb+t, c*128+p]
        xt = np.ascontiguousarray(
            shard.reshape(ntok // tb, tb, NB, P).transpose(0, 2, 3, 1),
            dtype=np.float16,
        )
        in_maps.append(
            {
                "xT": xt,
                "Bt": B_resh,
                "At": A_scaled,
            }
        )
    return in_maps, ntok


def kernel_with_results(x, lora_A, lora_B, trace=False, **kwargs):
    from concourse.bass_utils import run_bass_kernel_spmd

    in_maps, ntok = make_in_maps(x, lora_A, lora_B)
    nc = build_nc(ntok, tb=TB)
    res = run_bass_kernel_spmd(nc, in_maps, list(range(N_CORES)), trace=trace, **kwargs)
    out = np.concatenate([r["out"] for r in res.results], axis=0)
    return out.reshape(np.asarray(x).shape[:-1] + (OUT,)), res


def kernel(x, lora_A, lora_B):
    out, _ = kernel_with_results(x, lora_A, lora_B)
    return out
